# revision 65
# baseline (speedup 1.0000x reference)
"""Bass/Tile kernel for causal MHA block (nn_BlankAttention), bf16 matmuls.

Sharding: 8 cores = 2 batches x 4 head-groups (4 heads each); host sums the
4 per-batch partial outputs (w_out is row-sharded).

Structure per core:
  Phase A/B (per tok-chunk tk): QK projection (8 ft groups x 16-deep PSUM
    accumulation) + V projection -- x streamed once, dense PE; V tiles 12-15
    deferred into phase C as PE filler; PE clock pre-warmed with
    dependency-free matmuls during the DMA ramp.
  Phase C: attention per l-tile i ascending, heads processed in PAIRS (two
    j-loop passes): one 2-bank scores tile -> one paired exp (ACT) -> paired
    mask-mul (DVE), scores software-pipelined one j-group ahead of the AV
    matmuls; softmax denominators via column-tiled M=1 matmuls (concurrent in
    one PSUM bank); y-projection groups of already-normed l-tiles fill the PE
    during exp stalls, with dependency-free keep-warm matmuls as fallback so
    the HAM clock-gate stays at full rate.
  Tail: last l-tile's norm (reciprocal_approx_fast) + its y-proj groups in a
    fresh deep-buffered PSUM pool; y is written as bf16 and upcast on host.

All inputs are pre-arranged on the host into [128-partition, free] layouts so
every DMA moves >=4KB contiguous per partition.
"""

import numpy as np
import ml_dtypes
import concourse.bass as bass
import concourse.tile as tile
from concourse import bacc, mybir

S = 2048
DM = 2048
NHL = 4          # heads per core
DH = 128
SCALE = 1.0 / (DH ** 0.5)

F32 = mybir.dt.float32
F32R = mybir.dt.float32r
BF16 = mybir.dt.bfloat16
NPBF16 = ml_dtypes.bfloat16
EXP = mybir.ActivationFunctionType.Exp


def build_nc(schedule, n_masks):
    nc = bacc.Bacc("TRN2", target_bir_lowering=False, debug=False, num_devices=8)
    # host-prearranged layouts (partition dim second-to-... see make_core_inputs)
    xt_d = nc.dram_tensor("xt", [4, 128, 16, 512], BF16, kind="ExternalInput").ap()
    wqk_d = nc.dram_tensor("wqk", [8, 128, 16, 128], BF16, kind="ExternalInput").ap()
    wv_d = nc.dram_tensor("wv", [128, 16, 512], BF16, kind="ExternalInput").ap()
    wout_d = nc.dram_tensor("wout", [128, 4, S], BF16, kind="ExternalInput").ap()
    maskt_d = nc.dram_tensor("maskt", [128, n_masks, 2, 512], BF16, kind="ExternalInput").ap()
    ones_d = nc.dram_tensor("ones", [128, 128], BF16, kind="ExternalInput").ap()
    y_d = nc.dram_tensor("y", [S, DM], BF16, kind="ExternalOutput").ap()

    with tile.TileContext(nc) as tc:
        with tc.tile_pool(name="persist", bufs=1) as pp:
            qkT = pp.tile([128, 8, S], BF16)      # [dh, (2h+isK), tok]
            V = pp.tile([128, 16, 512], BF16)     # [tok%128, tok//128, vfeat]
            masks = pp.tile([128, n_masks, 2, 512], BF16)
            ones_t = pp.tile([128, 128], BF16)
            OT = pp.tile([128, 4, S], BF16)       # [dh, h, tok]
            woutT = pp.tile([128, 4, S], BF16)    # [dh, h, od]
            # x chunk 3 + wv persist past the proj pool: V tiles 12-15 are
            # computed as filler during i=0's attention (only l-tile 3 reads
            # them)
            xc3 = pp.tile([128, 16, 512], BF16)
            wv_sb = pp.tile([128, 16, 512], BF16)

            # ---- Phase A/B: QK + V projections, x streamed once ----
            with tc.tile_pool(name="proj", bufs=1) as projp, \
                 tc.tile_pool(name="pps", bufs=1, space="PSUM") as pps:
                nc.sync.dma_start(ones_t[:], ones_d[:])
                wqk_fts = []
                for ft in range(1):
                    w = projp.tile([128, 16, 128], BF16, tag="wqk", bufs=8,
                                   name=f"wqk{ft}")
                    nc.sync.dma_start(w[:], wqk_d[ft])
                    wqk_fts.append(w)
                xcs = []
                for tk in range(4):
                    xc = projp.tile([128, 16, 512], BF16, tag="xt", bufs=2,
                                    name=f"xt{tk}")
                    if tk == 0:
                        for qq in range(8):
                            nc.sync.dma_start(xc[:, 2 * qq:2 * (qq + 1), :],
                                              xt_d[0, :, 2 * qq:2 * (qq + 1), :])
                    xcs.append(xc)
                for ft in range(1, 8):
                    w = projp.tile([128, 16, 128], BF16, tag="wqk", bufs=8,
                                   name=f"wqk{ft}")
                    nc.sync.dma_start(w[:], wqk_d[ft])
                    wqk_fts.append(w)
                nc.sync.dma_start(wv_sb[:], wv_d[:])
                nc.sync.dma_start(masks[:], maskt_d[:])
                nc.sync.dma_start(woutT[:], wout_d[:])

                # pre-warm the PE clock (HAM) during the DMA ramp with
                # dependency-free matmuls on the ones tile
                warm = pps.tile([128, 512], F32, tag="ps", bufs=4, name="warm")
                for _ in range(45):
                    nc.tensor.matmul(warm[:, 0:128], ones_t[0:1, :],
                                     ones_t[0:1, :], start=True, stop=True)

                for tk in range(4):
                    xc = xcs[tk] if tk < 3 else xc3
                    if tk > 0:
                        for qq in range(4):
                            nc.sync.dma_start(xc[:, 4 * qq:4 * (qq + 1), :],
                                              xt_d[tk, :, 4 * qq:4 * (qq + 1), :])
                    for ft in range(8):
                        ps = pps.tile([128, 512], F32, tag="ps", bufs=4,
                                      name=f"ps{tk}_{ft}")
                        for dq in range(16):
                            nc.tensor.matmul(
                                ps[:], wqk_fts[ft][:, dq, :], xc[:, dq, :],
                                start=(dq == 0), stop=(dq == 15))
                        nc.vector.tensor_copy(qkT[:, ft, 512 * tk:512 * (tk + 1)],
                                              ps[:])
                    for tl in range(4):
                        tt = 4 * tk + tl
                        if tt >= 12:
                            continue  # deferred: V-filler during i=0 attention
                        ps2 = pps.tile([128, 512], F32, tag="ps", bufs=4,
                                       name=f"psv{tt}")
                        for dq in range(16):
                            nc.tensor.matmul(
                                ps2[:], xc[:, dq, 128 * tl:128 * (tl + 1)],
                                wv_sb[:, dq, :],
                                start=(dq == 0), stop=(dq == 15))
                        nc.scalar.copy(V[:, tt, :], ps2[:])

            # ---- Phase C: attention + interleaved output projection ----
            from collections import deque
            filler = deque()
            with tc.tile_pool(name="attn", bufs=1) as ap:

                def make_group(tt, o, on_act, pool, bufs):
                    def g():
                        yp = pool.tile([128, 512], F32, tag="yp", bufs=bufs,
                                       name=f"yp{tt}_{o}")
                        for h in range(4):
                            nc.tensor.matmul(
                                yp[:], OT[:, h, 128 * tt:128 * (tt + 1)],
                                woutT[:, h, 512 * o:512 * (o + 1)],
                                start=(h == 0), stop=(h == 3))
                        ys = ap.tile([128, 512], BF16, tag="ys", bufs=4,
                                     name=f"ys{tt}_{o}")
                        if on_act:
                            nc.scalar.copy(ys[:], yp[:])
                        else:
                            nc.vector.tensor_copy(ys[:], yp[:])
                        nc.sync.dma_start(
                            y_d[128 * tt:128 * (tt + 1), 512 * o:512 * (o + 1)],
                            ys[:])
                    return g

                with tc.tile_pool(name="aps", bufs=1, space="PSUM") as aps:
                    oacc2 = aps.tile([128, 2, 512], F32)   # AV accum, banks 0-1
                    s4 = aps.tile([128, 512], F32)         # sums rows 32h, bank 2

                    def v_unit(tt):
                        # deferred V-projection tile: dense PE filler for i=0
                        def g():
                            ps2 = aps.tile([128, 512], F32, tag="yp", bufs=1,
                                           name=f"vps{tt}")
                            for dq in range(16):
                                nc.tensor.matmul(
                                    ps2[:],
                                    xc3[:, dq, 128 * (tt % 4):128 * (tt % 4 + 1)],
                                    wv_sb[:, dq, :],
                                    start=(dq == 0), stop=(dq == 15))
                            nc.scalar.copy(V[:, tt, :], ps2[:])
                        return g

                    vq = deque(v_unit(tt) for tt in range(12, 16))

                    for i in range(4):
                        js = schedule[i]
                        nj = len(js)
                        # i=0: force full-width (lo=0) so every sums parity
                        # row is fully written; masks zero the dead prefix

                        def scores_pair(pas, jidx):
                            # paired scores: 2 heads in one 2-bank tile,
                            # one exp + one mask-mul for both
                            j, mi, lo = js[jidx]
                            if i == 0:
                                lo = 0
                            scp = aps.tile([128, 2, 512], F32, tag="sc",
                                           bufs=2, name=f"sc{i}_{pas}_{j}")
                            for k in range(2):
                                h = 2 * pas + k
                                nc.tensor.matmul(
                                    scp[:, k, lo:],
                                    qkT[:, 2 * h + 1, 128 * j:128 * (j + 1)],
                                    qkT[:, 2 * h, 512 * i + lo:512 * (i + 1)],
                                    start=True, stop=True)
                            ex = ap.tile([128, 2, 512], BF16, tag="ex",
                                         bufs=6, name=f"ex{i}_{pas}_{j}")
                            nc.scalar.activation(ex[:, :, lo:], scp[:, :, lo:],
                                                 EXP, scale=SCALE)
                            if mi >= 0:
                                nc.vector.tensor_mul(ex[:, :, lo:],
                                                     ex[:, :, lo:],
                                                     masks[:, mi, :, lo:])
                            return ex

                        def sums_mm(ex_t, lo, col, first, last):
                            nc.tensor.matmul(
                                s4[col:col + 1, lo:], ones_t[:, 0:1],
                                ex_t[:, (col // 32) % 2, lo:],
                                start=first, stop=last,
                                tile_position=(0, col),
                                skip_group_check=True)

                        pend = None
                        for pas in range(2):
                            # scores pipelined one j-group ahead of AV
                            exq = scores_pair(pas, 0)
                            if vq:
                                vq.popleft()()
                            elif filler and (i < 3 or len(filler) > 6):
                                filler.popleft()()
                            held = None
                            for jidx, (j, mi, lo) in enumerate(js):
                                if i == 0:
                                    lo = 0
                                ex_next = (scores_pair(pas, jidx + 1)
                                           if jidx + 1 < nj else None)
                                for k in range(2):
                                    h = 2 * pas + k
                                    nc.tensor.matmul(
                                        oacc2[:, k, lo:],
                                        V[:, j, 128 * h:128 * (h + 1)],
                                        exq[:, k, lo:],
                                        start=(jidx == 0), stop=(jidx == nj - 1))
                                # deferred norm-half of the previous pass runs
                                # here, once its reciprocal chain is long done
                                if jidx == 1 and pend is not None:
                                    pend()
                                    pend = None
                                # sums: hold even j-groups; emit a 4-wide
                                # column-tiled quad (both parities, both
                                # heads) on odd j-groups -- 4 concurrent MMs
                                par = jidx % 2
                                if par == 0:
                                    held = (exq, lo)
                                    if jidx == nj - 1:  # odd nj fallback
                                        for k in range(2):
                                            sums_mm(exq, lo, 32 * k,
                                                    jidx == 0, True)
                                else:
                                    hex_t, hlo = held
                                    last = jidx == nj - 1
                                    for k in range(2):
                                        sums_mm(hex_t, hlo, 32 * k,
                                                jidx == 1, last)
                                    for k in range(2):
                                        sums_mm(exq, lo, 64 + 32 * k,
                                                jidx == 1, last)
                                    held = None
                                exq = ex_next
                                # paced filler: hold 6 units for the last norm
                                if vq and jidx % 2 == 1:
                                    vq.popleft()()
                                elif filler and jidx % 2 == 1 and \
                                        (i < 3 or len(filler) > 6):
                                    filler.popleft()()
                                elif not filler and jidx % 2 == 1:
                                    # keep-warm: dependency-free matmul keeps
                                    # the PE clock hot through exp stalls
                                    dum = aps.tile([128, 512], F32, tag="yp",
                                                   bufs=1, name=f"dm{i}{pas}{jidx}")
                                    nc.tensor.matmul(dum[:], ones_t[0:1, :],
                                                     qkT[0:1, 0, 0:512],
                                                     start=True, stop=True)
                            # row-pair reduce + reciprocal for this pass's two
                            # heads: parity rows 0/64 (k=0) and 32/96 (k=1)
                            shi = ap.tile([64, 512], F32, tag="shi", bufs=2,
                                          name=f"shi{i}_{pas}")
                            nc.scalar.copy(shi[:], s4[64:128, :])
                            ssum = ap.tile([64, 512], F32, tag="ssum", bufs=2,
                                           name=f"ssum{i}_{pas}")
                            nc.vector.tensor_add(ssum[:], s4[0:64, :], shi[:])
                            rec_p = ap.tile([64, 512], F32, tag="rec", bufs=2,
                                            name=f"rec{i}_{pas}")
                            nc.vector.reciprocal_approx_fast(rec_p[:], ssum[:])
                            # evacuate this pass's AV accumulators (ACT+DVE)
                            o_sbs2 = []
                            for k in range(2):
                                o_sb = ap.tile([128, 512], F32, tag="osb", bufs=5,
                                               name=f"osb{i}_{2 * pas + k}")
                                if k == 0:
                                    nc.scalar.copy(o_sb[:], oacc2[:, k, :])
                                else:
                                    nc.vector.tensor_copy(o_sb[:], oacc2[:, k, :])
                                o_sbs2.append(o_sb)

                            def norm_half(pas_, rec_, o_sbs_):
                                def g():
                                    for k in range(2):
                                        h = 2 * pas_ + k
                                        rtmp = ap.tile([1, 512], BF16,
                                                       tag="rtmp", bufs=4,
                                                       name=f"rt{i}_{h}")
                                        nc.vector.tensor_copy(
                                            rtmp[:], rec_[32 * k:32 * k + 1, :])
                                        bc = aps.tile([128, 2, 512], F32,
                                                      tag="sc", bufs=2,
                                                      name=f"bc{i}_{h}")
                                        if i == 3:
                                            for _ in range(2):
                                                nc.tensor.matmul(
                                                    bc[:, 1, :], ones_t[0:1, :],
                                                    qkT[0:1, 0, 0:512],
                                                    start=True, stop=True)
                                        nc.tensor.matmul(
                                            bc[:, 0, :], ones_t[0:1, :],
                                            rtmp[:], start=True, stop=True)
                                        nc.vector.tensor_mul(
                                            OT[:, h, 512 * i:512 * (i + 1)],
                                            o_sbs_[k][:], bc[:, 0, :])
                                return g

                            pend = norm_half(pas, rec_p, o_sbs2)

                        # last pass's norm-half runs at the l-tile boundary
                        if i == 3:
                            while filler:
                                filler.popleft()()
                        pend()
                        pend = None
                        if filler:
                            filler.popleft()()

                        if i < 3:
                            filler.extend(make_group(tt, o, False, aps, 1)
                                          for tt in range(4 * i, 4 * i + 4)
                                          for o in range(4))

                # tail: last l-tile's groups in a fresh deep PSUM pool
                with tc.tile_pool(name="aps2", bufs=1, space="PSUM") as aps2:
                    for tt in range(12, 16):
                        for o in range(4):
                            make_group(tt, o, o % 2 == 0, aps2, 6)()
    nc.compile()
    return nc


def derive_schedule(mask):
    """mask: [S, S] bool, True = masked. Returns (schedule, mask_tiles)."""
    schedule = []
    uniq = {}
    tiles = []
    for i in range(4):
        row = []
        for j in range(16):
            blk = mask[512 * i:512 * (i + 1), 128 * j:128 * (j + 1)]
            if blk.all():
                continue
            if not blk.any():
                row.append((j, -1, 0))
                continue
            t = (~blk.T).astype(np.float32)  # [L 128, l 512], allowed=1
            nz = np.flatnonzero(t.any(axis=0))
            lo = min(int(nz[0]) if len(nz) else 0, 256)
            key = t.tobytes()
            if key not in uniq:
                uniq[key] = len(tiles)
                tiles.append(t)
            row.append((j, uniq[key], lo))
        schedule.append(row)
    if not tiles:
        tiles.append(np.ones((128, 512), np.float32))
    return schedule, np.stack(tiles)


def _part(a, t):
    """[t*128, f] -> [128, t, f] partition-major prearrangement."""
    f = a.shape[1]
    return np.ascontiguousarray(a.reshape(t, 128, f).transpose(1, 0, 2))


def make_core_inputs(x, w_in, w_out, mask_tiles, b, hg):
    """Inputs for core handling batch b, heads hg*4..hg*4+3 (prearranged)."""
    heads = range(hg * 4, hg * 4 + 4)
    xt = np.ascontiguousarray(x[b].T).astype(NPBF16)         # [DM, S]
    wqk = np.concatenate(
        [w_in[:, h * 384 + o:h * 384 + o + 128] for h in heads for o in (0, 128)],
        axis=1).astype(NPBF16)                               # [DM, 1024]
    wv = np.concatenate([w_in[:, h * 384 + 256:h * 384 + 384] for h in heads],
                        axis=1).astype(NPBF16)               # [DM, 512]
    wout = np.concatenate([w_out[h * 128:(h + 1) * 128, :] for h in heads],
                          axis=0).astype(NPBF16)             # [512, DM]
    xt_pre = np.stack([_part(xt[:, 512 * tk:512 * (tk + 1)], 16)
                       for tk in range(4)])                  # [4,128,16,512]
    wqk_pre = np.stack([_part(wqk[:, 128 * ft:128 * (ft + 1)], 16)
                        for ft in range(8)])                 # [8,128,16,128]
    wv_pre = _part(wv, 16)                                   # [128,16,512]
    wout_pre = _part(wout, 4)                                # [128,4,2048]
    mt = mask_tiles.astype(NPBF16).transpose(1, 0, 2)        # [128,n,512]
    maskt_pre = np.ascontiguousarray(
        np.broadcast_to(mt[:, :, None, :],
                        (128, mt.shape[1], 2, 512)))         # [128,n,2,512]
    return {
        "xt": xt_pre,
        "wqk": wqk_pre,
        "wv": wv_pre,
        "wout": wout_pre,
        "maskt": maskt_pre,
        "ones": np.ones((128, 128), NPBF16),
    }


_CACHE = {}


def _get_nc(schedule, n_masks):
    key = (tuple(tuple(r) for r in schedule), n_masks)
    if key not in _CACHE:
        _CACHE[key] = build_nc(schedule, n_masks)
    return _CACHE[key]


def kernel(x, w_in, w_out, mask):
    """Full-input entry point: shards across 8 NeuronCores (batch x head-group),
    runs the Bass kernel SPMD, and reduces the per-core partial outputs."""
    from concourse import bass_utils
    x = np.ascontiguousarray(np.asarray(x), dtype=np.float32)
    w_in = np.ascontiguousarray(np.asarray(w_in), dtype=np.float32)
    w_out = np.ascontiguousarray(np.asarray(w_out), dtype=np.float32)
    B = x.shape[0]
    m2 = np.asarray(mask).reshape(S, S)
    schedule, mask_tiles = derive_schedule(m2)
    nc = _get_nc(schedule, mask_tiles.shape[0])
    in_maps = [make_core_inputs(x, w_in, w_out, mask_tiles, c // 4, c % 4)
               for c in range(8)]
    res = bass_utils.run_bass_kernel_spmd(nc, in_maps, core_ids=list(range(8)))
    y = np.zeros((B, S, DM), np.float32)
    for c in range(8):
        y[c // 4] += np.asarray(res.results[c]["y"]).astype(np.float32)
    return y


# revision 67
# speedup vs baseline: 1.0252x; 1.0252x over previous
"""Bass/Tile kernel for causal MHA block (nn_BlankAttention), bf16 matmuls.

Sharding: 8 cores = 2 batches x 4 head-groups (4 heads each); host sums the
4 per-batch partial outputs (w_out is row-sharded).

Structure per core:
  Phase A/B (per tok-chunk tk): QK projection (8 ft groups x 16-deep PSUM
    accumulation) + V projection -- x streamed once, dense PE; V tiles 12-15
    deferred into phase C as PE filler; PE clock pre-warmed with
    dependency-free matmuls during the DMA ramp.
  Phase C: attention per l-tile i ascending, heads processed in PAIRS (two
    j-loop passes): one 2-bank scores tile -> one paired exp (ACT) -> paired
    mask-mul (DVE), scores software-pipelined one j-group ahead of the AV
    matmuls; softmax denominators via column-tiled M=1 matmuls (concurrent in
    one PSUM bank); y-projection groups of already-normed l-tiles fill the PE
    during exp stalls, with dependency-free keep-warm matmuls as fallback so
    the HAM clock-gate stays at full rate.
  Tail: last l-tile's norm (reciprocal_approx_fast) + its y-proj groups in a
    fresh deep-buffered PSUM pool; y is written as bf16 and upcast on host.

All inputs are pre-arranged on the host into [128-partition, free] layouts so
every DMA moves >=4KB contiguous per partition.
"""

import numpy as np
import ml_dtypes
import concourse.bass as bass
import concourse.tile as tile
from concourse import bacc, mybir

S = 2048
DM = 2048
NHL = 4          # heads per core
DH = 128
SCALE = 1.0 / (DH ** 0.5)

F32 = mybir.dt.float32
F32R = mybir.dt.float32r
BF16 = mybir.dt.bfloat16
NPBF16 = ml_dtypes.bfloat16
EXP = mybir.ActivationFunctionType.Exp


def build_nc(schedule, n_masks):
    nc = bacc.Bacc("TRN2", target_bir_lowering=False, debug=False, num_devices=8)
    # host-prearranged layouts (partition dim second-to-... see make_core_inputs)
    xt_d = nc.dram_tensor("xt", [4, 128, 16, 512], BF16, kind="ExternalInput").ap()
    wqk_d = nc.dram_tensor("wqk", [8, 128, 16, 128], BF16, kind="ExternalInput").ap()
    wv_d = nc.dram_tensor("wv", [128, 16, 512], BF16, kind="ExternalInput").ap()
    wout_d = nc.dram_tensor("wout", [128, 4, S], BF16, kind="ExternalInput").ap()
    maskt_d = nc.dram_tensor("maskt", [128, n_masks, 2, 512], BF16, kind="ExternalInput").ap()
    ones_d = nc.dram_tensor("ones", [128, 128], BF16, kind="ExternalInput").ap()
    y_d = nc.dram_tensor("y", [S, DM], BF16, kind="ExternalOutput").ap()

    with tile.TileContext(nc) as tc:
        with tc.tile_pool(name="persist", bufs=1) as pp:
            qkT = pp.tile([128, 8, S], BF16)      # [dh, (2h+isK), tok]
            V = pp.tile([128, 16, 512], BF16)     # [tok%128, tok//128, vfeat]
            masks = pp.tile([128, n_masks, 2, 512], BF16)
            ones_t = pp.tile([128, 128], BF16)
            OT = pp.tile([128, 4, S], BF16)       # [dh, h, tok]
            woutT = pp.tile([128, 4, S], BF16)    # [dh, h, od]
            # x chunk 3 + wv persist past the proj pool: V tiles 12-15 are
            # computed as filler during i=0's attention (only l-tile 3 reads
            # them)
            xc3 = pp.tile([128, 16, 512], BF16)
            wv_sb = pp.tile([128, 16, 512], BF16)

            # ---- Phase A/B: QK + V projections, x streamed once ----
            with tc.tile_pool(name="proj", bufs=1) as projp, \
                 tc.tile_pool(name="pps", bufs=1, space="PSUM") as pps:
                nc.sync.dma_start(ones_t[:], ones_d[:])
                wqk_fts = []
                for ft in range(1):
                    w = projp.tile([128, 16, 128], BF16, tag="wqk", bufs=8,
                                   name=f"wqk{ft}")
                    nc.sync.dma_start(w[:], wqk_d[ft])
                    wqk_fts.append(w)
                xcs = []
                for tk in range(4):
                    xc = projp.tile([128, 16, 512], BF16, tag="xt", bufs=2,
                                    name=f"xt{tk}")
                    if tk == 0:
                        for qq in range(8):
                            nc.sync.dma_start(xc[:, 2 * qq:2 * (qq + 1), :],
                                              xt_d[0, :, 2 * qq:2 * (qq + 1), :])
                    xcs.append(xc)
                for ft in range(1, 8):
                    w = projp.tile([128, 16, 128], BF16, tag="wqk", bufs=8,
                                   name=f"wqk{ft}")
                    nc.sync.dma_start(w[:], wqk_d[ft])
                    wqk_fts.append(w)
                nc.sync.dma_start(wv_sb[:], wv_d[:])
                nc.sync.dma_start(masks[:], maskt_d[:])
                nc.sync.dma_start(woutT[:], wout_d[:])

                # pre-warm the PE clock (HAM) during the DMA ramp with
                # dependency-free matmuls on the ones tile
                warm = pps.tile([128, 512], F32, tag="ps", bufs=4, name="warm")
                for _ in range(45):
                    nc.tensor.matmul(warm[:, 0:128], ones_t[0:1, :],
                                     ones_t[0:1, :], start=True, stop=True)

                for tk in range(4):
                    xc = xcs[tk] if tk < 3 else xc3
                    if tk > 0:
                        for qq in range(4):
                            nc.sync.dma_start(xc[:, 4 * qq:4 * (qq + 1), :],
                                              xt_d[tk, :, 4 * qq:4 * (qq + 1), :])
                    for ft in range(8):
                        ps = pps.tile([128, 512], F32, tag="ps", bufs=4,
                                      name=f"ps{tk}_{ft}")
                        for dq in range(16):
                            nc.tensor.matmul(
                                ps[:], wqk_fts[ft][:, dq, :], xc[:, dq, :],
                                start=(dq == 0), stop=(dq == 15))
                        nc.vector.tensor_copy(qkT[:, ft, 512 * tk:512 * (tk + 1)],
                                              ps[:])
                    for tl in range(4):
                        tt = 4 * tk + tl
                        if tt >= 12:
                            continue  # deferred: V-filler during i=0 attention
                        ps2 = pps.tile([128, 512], F32, tag="ps", bufs=4,
                                       name=f"psv{tt}")
                        for dq in range(16):
                            nc.tensor.matmul(
                                ps2[:], xc[:, dq, 128 * tl:128 * (tl + 1)],
                                wv_sb[:, dq, :],
                                start=(dq == 0), stop=(dq == 15))
                        nc.scalar.copy(V[:, tt, :], ps2[:])

            # ---- Phase C: attention + interleaved output projection ----
            from collections import deque
            filler = deque()
            with tc.tile_pool(name="attn", bufs=1) as ap:

                def make_group(tt, o, on_act, pool, bufs):
                    def g():
                        yp = pool.tile([128, 512], F32, tag="yp", bufs=bufs,
                                       name=f"yp{tt}_{o}")
                        for h in range(4):
                            nc.tensor.matmul(
                                yp[:], OT[:, h, 128 * tt:128 * (tt + 1)],
                                woutT[:, h, 512 * o:512 * (o + 1)],
                                start=(h == 0), stop=(h == 3))
                        ys = ap.tile([128, 512], BF16, tag="ys", bufs=4,
                                     name=f"ys{tt}_{o}")
                        if on_act:
                            nc.scalar.copy(ys[:], yp[:])
                        else:
                            nc.vector.tensor_copy(ys[:], yp[:])
                        nc.sync.dma_start(
                            y_d[128 * tt:128 * (tt + 1), 512 * o:512 * (o + 1)],
                            ys[:])
                    return g

                with tc.tile_pool(name="aps", bufs=1, space="PSUM") as aps:
                    oacc2 = aps.tile([128, 2, 512], F32)   # AV accum, banks 0-1
                    s4 = aps.tile([128, 512], F32)         # sums rows 32h, bank 2

                    def v_unit(tt):
                        # deferred V-projection tile: dense PE filler for i=0
                        def g():
                            ps2 = aps.tile([128, 512], F32, tag="yp", bufs=1,
                                           name=f"vps{tt}")
                            for dq in range(16):
                                nc.tensor.matmul(
                                    ps2[:],
                                    xc3[:, dq, 128 * (tt % 4):128 * (tt % 4 + 1)],
                                    wv_sb[:, dq, :],
                                    start=(dq == 0), stop=(dq == 15))
                            nc.scalar.copy(V[:, tt, :], ps2[:])
                        return g

                    vq = deque(v_unit(tt) for tt in range(12, 16))

                    for i in range(4):
                        js = schedule[i]
                        nj = len(js)
                        # i=0: force full-width (lo=0) so every sums parity
                        # row is fully written; masks zero the dead prefix

                        def scores_pair(pas, jidx):
                            # paired scores: 2 heads in one 2-bank tile,
                            # one exp + one mask-mul for both
                            j, mi, lo = js[jidx]
                            if i == 0:
                                lo = 0
                            scp = aps.tile([128, 2, 512], F32, tag="sc",
                                           bufs=2, name=f"sc{i}_{pas}_{j}")
                            for k in range(2):
                                h = 2 * pas + k
                                nc.tensor.matmul(
                                    scp[:, k, lo:],
                                    qkT[:, 2 * h + 1, 128 * j:128 * (j + 1)],
                                    qkT[:, 2 * h, 512 * i + lo:512 * (i + 1)],
                                    start=True, stop=True)
                            ex = ap.tile([128, 2, 512], BF16, tag="ex",
                                         bufs=6, name=f"ex{i}_{pas}_{j}")
                            nc.scalar.activation(ex[:, :, lo:], scp[:, :, lo:],
                                                 EXP, scale=SCALE)
                            if mi >= 0:
                                nc.vector.tensor_mul(ex[:, :, lo:],
                                                     ex[:, :, lo:],
                                                     masks[:, mi, :, lo:])
                            return ex

                        def sums_mm(ex_t, lo, col, first, last):
                            nc.tensor.matmul(
                                s4[col:col + 1, lo:], ones_t[:, 0:1],
                                ex_t[:, (col // 32) % 2, lo:],
                                start=first, stop=last,
                                tile_position=(0, col),
                                skip_group_check=True)

                        pend = None
                        for pas in range(2):
                            # scores pipelined one j-group ahead of AV
                            exq = scores_pair(pas, 0)
                            if vq:
                                vq.popleft()()
                            elif filler and (i < 3 or len(filler) > 6):
                                filler.popleft()()
                            held = []
                            for jidx, (j, mi, lo) in enumerate(js):
                                if i == 0:
                                    lo = 0
                                ex_next = (scores_pair(pas, jidx + 1)
                                           if jidx + 1 < nj else None)
                                # sums run two groups behind: both quad inputs
                                # are then long done, so the 4 column-tiled
                                # MMs genuinely overlap in the PE array
                                if len(held) == 2:
                                    for k in range(2):
                                        sums_mm(held[0][0], held[0][1], 32 * k,
                                                jidx == 2, False)
                                    for k in range(2):
                                        sums_mm(held[1][0], held[1][1],
                                                64 + 32 * k, jidx == 2, False)
                                    held = []
                                for k in range(2):
                                    h = 2 * pas + k
                                    nc.tensor.matmul(
                                        oacc2[:, k, lo:],
                                        V[:, j, 128 * h:128 * (h + 1)],
                                        exq[:, k, lo:],
                                        start=(jidx == 0), stop=(jidx == nj - 1))
                                # deferred norm-half of the previous pass runs
                                # here, once its reciprocal chain is long done
                                if jidx == 2 and pend is not None:
                                    pend()
                                    pend = None
                                held.append((exq, lo))
                                exq = ex_next
                                # paced filler: hold 6 units for the last norm
                                if vq and jidx % 2 == 1:
                                    vq.popleft()()
                                elif filler and jidx % 2 == 1 and \
                                        (i < 3 or len(filler) > 6):
                                    filler.popleft()()
                                elif not filler and jidx % 2 == 1:
                                    # keep-warm: dependency-free matmul keeps
                                    # the PE clock hot through exp stalls
                                    dum = aps.tile([128, 512], F32, tag="yp",
                                                   bufs=1, name=f"dm{i}{pas}{jidx}")
                                    nc.tensor.matmul(dum[:], ones_t[0:1, :],
                                                     qkT[0:1, 0, 0:512],
                                                     start=True, stop=True)
                            # flush the last two held groups' sums
                            for k in range(2):
                                sums_mm(held[0][0], held[0][1], 32 * k,
                                        False, True)
                            for k in range(2):
                                sums_mm(held[1][0], held[1][1], 64 + 32 * k,
                                        False, True)
                            held = []
                            # row-pair reduce + reciprocal for this pass's two
                            # heads: parity rows 0/64 (k=0) and 32/96 (k=1)
                            shi = ap.tile([64, 512], F32, tag="shi", bufs=2,
                                          name=f"shi{i}_{pas}")
                            nc.scalar.copy(shi[:], s4[64:128, :])
                            ssum = ap.tile([64, 512], F32, tag="ssum", bufs=2,
                                           name=f"ssum{i}_{pas}")
                            nc.vector.tensor_add(ssum[:], s4[0:64, :], shi[:])
                            rec_p = ap.tile([64, 512], F32, tag="rec", bufs=2,
                                            name=f"rec{i}_{pas}")
                            nc.vector.reciprocal_approx_fast(rec_p[:], ssum[:])
                            # evacuate this pass's AV accumulators (ACT+DVE)
                            o_sbs2 = []
                            for k in range(2):
                                o_sb = ap.tile([128, 512], F32, tag="osb", bufs=5,
                                               name=f"osb{i}_{2 * pas + k}")
                                if k == 0:
                                    nc.scalar.copy(o_sb[:], oacc2[:, k, :])
                                else:
                                    nc.vector.tensor_copy(o_sb[:], oacc2[:, k, :])
                                o_sbs2.append(o_sb)

                            def norm_half(pas_, rec_, o_sbs_):
                                def g():
                                    for k in range(2):
                                        h = 2 * pas_ + k
                                        rtmp = ap.tile([1, 512], BF16,
                                                       tag="rtmp", bufs=4,
                                                       name=f"rt{i}_{h}")
                                        nc.vector.tensor_copy(
                                            rtmp[:], rec_[32 * k:32 * k + 1, :])
                                        bc = aps.tile([128, 2, 512], F32,
                                                      tag="sc", bufs=2,
                                                      name=f"bc{i}_{h}")
                                        if i == 3:
                                            for _ in range(2):
                                                nc.tensor.matmul(
                                                    bc[:, 1, :], ones_t[0:1, :],
                                                    qkT[0:1, 0, 0:512],
                                                    start=True, stop=True)
                                        nc.tensor.matmul(
                                            bc[:, 0, :], ones_t[0:1, :],
                                            rtmp[:], start=True, stop=True)
                                        nc.vector.tensor_mul(
                                            OT[:, h, 512 * i:512 * (i + 1)],
                                            o_sbs_[k][:], bc[:, 0, :])
                                return g

                            pend = norm_half(pas, rec_p, o_sbs2)

                        # last pass's norm-half runs at the l-tile boundary
                        if i == 3:
                            while filler:
                                filler.popleft()()
                        pend()
                        pend = None
                        if filler:
                            filler.popleft()()

                        if i < 3:
                            filler.extend(make_group(tt, o, False, aps, 1)
                                          for tt in range(4 * i, 4 * i + 4)
                                          for o in range(4))

                # tail: last l-tile's groups in a fresh deep PSUM pool
                with tc.tile_pool(name="aps2", bufs=1, space="PSUM") as aps2:
                    for tt in range(12, 16):
                        for o in range(4):
                            make_group(tt, o, o % 2 == 0, aps2, 6)()
    nc.compile()
    return nc


def derive_schedule(mask):
    """mask: [S, S] bool, True = masked. Returns (schedule, mask_tiles)."""
    schedule = []
    uniq = {}
    tiles = []
    for i in range(4):
        row = []
        for j in range(16):
            blk = mask[512 * i:512 * (i + 1), 128 * j:128 * (j + 1)]
            if blk.all():
                continue
            if not blk.any():
                row.append((j, -1, 0))
                continue
            t = (~blk.T).astype(np.float32)  # [L 128, l 512], allowed=1
            nz = np.flatnonzero(t.any(axis=0))
            lo = min(int(nz[0]) if len(nz) else 0, 256)
            key = t.tobytes()
            if key not in uniq:
                uniq[key] = len(tiles)
                tiles.append(t)
            row.append((j, uniq[key], lo))
        schedule.append(row)
    if not tiles:
        tiles.append(np.ones((128, 512), np.float32))
    return schedule, np.stack(tiles)


def _part(a, t):
    """[t*128, f] -> [128, t, f] partition-major prearrangement."""
    f = a.shape[1]
    return np.ascontiguousarray(a.reshape(t, 128, f).transpose(1, 0, 2))


def make_core_inputs(x, w_in, w_out, mask_tiles, b, hg):
    """Inputs for core handling batch b, heads hg*4..hg*4+3 (prearranged)."""
    heads = range(hg * 4, hg * 4 + 4)
    xt = np.ascontiguousarray(x[b].T).astype(NPBF16)         # [DM, S]
    wqk = np.concatenate(
        [w_in[:, h * 384 + o:h * 384 + o + 128] for h in heads for o in (0, 128)],
        axis=1).astype(NPBF16)                               # [DM, 1024]
    wv = np.concatenate([w_in[:, h * 384 + 256:h * 384 + 384] for h in heads],
                        axis=1).astype(NPBF16)               # [DM, 512]
    wout = np.concatenate([w_out[h * 128:(h + 1) * 128, :] for h in heads],
                          axis=0).astype(NPBF16)             # [512, DM]
    xt_pre = np.stack([_part(xt[:, 512 * tk:512 * (tk + 1)], 16)
                       for tk in range(4)])                  # [4,128,16,512]
    wqk_pre = np.stack([_part(wqk[:, 128 * ft:128 * (ft + 1)], 16)
                        for ft in range(8)])                 # [8,128,16,128]
    wv_pre = _part(wv, 16)                                   # [128,16,512]
    wout_pre = _part(wout, 4)                                # [128,4,2048]
    mt = mask_tiles.astype(NPBF16).transpose(1, 0, 2)        # [128,n,512]
    maskt_pre = np.ascontiguousarray(
        np.broadcast_to(mt[:, :, None, :],
                        (128, mt.shape[1], 2, 512)))         # [128,n,2,512]
    return {
        "xt": xt_pre,
        "wqk": wqk_pre,
        "wv": wv_pre,
        "wout": wout_pre,
        "maskt": maskt_pre,
        "ones": np.ones((128, 128), NPBF16),
    }


_CACHE = {}


def _get_nc(schedule, n_masks):
    key = (tuple(tuple(r) for r in schedule), n_masks)
    if key not in _CACHE:
        _CACHE[key] = build_nc(schedule, n_masks)
    return _CACHE[key]


def kernel(x, w_in, w_out, mask):
    """Full-input entry point: shards across 8 NeuronCores (batch x head-group),
    runs the Bass kernel SPMD, and reduces the per-core partial outputs."""
    from concourse import bass_utils
    x = np.ascontiguousarray(np.asarray(x), dtype=np.float32)
    w_in = np.ascontiguousarray(np.asarray(w_in), dtype=np.float32)
    w_out = np.ascontiguousarray(np.asarray(w_out), dtype=np.float32)
    B = x.shape[0]
    m2 = np.asarray(mask).reshape(S, S)
    schedule, mask_tiles = derive_schedule(m2)
    nc = _get_nc(schedule, mask_tiles.shape[0])
    in_maps = [make_core_inputs(x, w_in, w_out, mask_tiles, c // 4, c % 4)
               for c in range(8)]
    res = bass_utils.run_bass_kernel_spmd(nc, in_maps, core_ids=list(range(8)))
    y = np.zeros((B, S, DM), np.float32)
    for c in range(8):
        y[c // 4] += np.asarray(res.results[c]["y"]).astype(np.float32)
    return y
